# revision 16
# baseline (speedup 1.0000x reference)
"""Depthwise 5x5 correlation (stride 1, pad 2) over X[4, 32, 512, 512] fp32,
with a single shared [5, 5] kernel, on 8 Trainium2 NeuronCores.

Strategy (pure data parallel): the 4*32 = 128 images are split 16 per core.
The input is zero-padded host-side to [516, 516], so on device the conv
decomposes per kernel column j:
    O[h, w] = sum_j C_j[h, w],   C_j[h, w] = sum_k B_j[k, h] X'[h + k, w + j]
where B_j is a banded-Toeplitz stationary matrix (B_j[k, m] = kernel[k - m,
j]); one TensorE matmul per (row-block, j), five j's accumulating into one
PSUM bank, the W shift folded into the rhs read offset. The operand path is
fp16 end to end (PSUM accumulates fp32); fp16's 2^-11 quantization is far
inside the 2e-2 gate (fp8 was measured at 4e-2 — dead).

H tiles into 4 blocks of 124 output rows (input rows q*124 + [0, 128)). The
16-row tails of all images run as block-diagonal "edge group" matmuls
packing 6 images per stationary (K = 6*20, M = 6*16): 15 matmuls replace 80.
The last group's matmuls run before the last image's uniform blocks so the
run doesn't end on edge work.

DMA facts this kernel is shaped around (measured): a descriptor covers one
SBUF-partition run x one contiguous DRAM run; the SWDGE queue dispatches
~36 ns/descriptor over 16 engines; HWDGE queues are fine for loads but
execute STORES on a single engine at ~330 ns/descriptor (poison). So:
 - loads ride the two HWDGE rings with block-interleaved DRAM layout
   x2[img, k, q, w] -> 4 KB descriptors, one DMA per image;
 - all stores ride SWDGE. Output DRAM layout y3[pair, p, i2, q, w] packs
   TWO images per partition run -> 8 KB descriptors, one store per pair;
 - the final image instead casts in 4 partition strips and stores each
   strip separately (31 descriptors apiece) so the run's last dependency
   chain is cast(0.2us) -> push -> ~1.1us dispatch.

A PE p-state warmup (20 dummy matmuls on a zeroed scratch tile) runs during
the initial DMA fill: the PE clock needs ~6 us of continuous work to ramp
0.65 -> 2.4 GHz, and real matmuls then run at full rate from the start.
"""

import numpy as np

import concourse.bacc as bacc
import concourse.bass as bass
import concourse.mybir as mybir
import concourse.tile as tile
from concourse.bass_utils import run_bass_kernel_spmd

F32 = mybir.dt.float32
F16 = mybir.dt.float16

N_CORES = 8
IMGS_PER_CORE = 16
H = W = 512
HP = H + 4
WP = W + 4
KS = 5

NB = 4           # uniform row blocks per image
MB = 124         # output rows per uniform block
ME = 16          # output rows in the edge block (rows 496..512)
KE = ME + KS - 1  # padded input rows the edge block reads

GMAX = 6         # edge-group packing factor
N_WARM = 20
NPAIR = IMGS_PER_CORE // 2

_CACHE = {}


def build_bands(kern):
    """kern: [5, 5] -> banded-Toeplitz stationaries, fp16.

    Returns (B, BE): B[128, 5, 124] uniform-block bands with
    B[k, j, m] = kern[k - m, j] for k - m in [0, 5); BE[120, 5, 96]
    block-diagonal edge bands packing GMAX images."""
    kern = np.asarray(kern, dtype=np.float32)
    B = np.zeros((MB + 4, KS, MB), dtype=np.float32)
    k_idx = np.arange(MB + 4)[:, None]
    m_idx = np.arange(MB)[None, :]
    tap = k_idx - m_idx
    valid = (tap >= 0) & (tap < KS)
    kk, mm = np.nonzero(valid)
    for j in range(KS):
        B[kk, j, mm] = kern[tap[kk, mm], j]

    BE = np.zeros((GMAX * KE, KS, GMAX * ME), dtype=np.float32)
    for g in range(GMAX):
        BE[g * KE:(g + 1) * KE, :, g * ME:(g + 1) * ME] = B[:KE, :, :ME]
    return B.astype(np.float16), BE.astype(np.float16)


def build_nc():
    nc = bacc.Bacc("TRN2", target_bir_lowering=False, debug=False)

    x2 = nc.dram_tensor(
        "x2", [IMGS_PER_CORE, 128, NB, WP], F16, kind="ExternalInput"
    ).ap()
    xe = nc.dram_tensor(
        "xe", [IMGS_PER_CORE, KE, WP], F16, kind="ExternalInput"
    ).ap()
    bm = nc.dram_tensor("bm", [MB + 4, KS, MB], F16, kind="ExternalInput").ap()
    bme = nc.dram_tensor(
        "bme", [GMAX * KE, KS, GMAX * ME], F16, kind="ExternalInput"
    ).ap()
    y3 = nc.dram_tensor(
        "y3", [NPAIR, MB, 2, NB, W], F16, kind="ExternalOutput"
    ).ap()
    y2e = nc.dram_tensor(
        "y2e", [IMGS_PER_CORE, ME, W], F16, kind="ExternalOutput"
    ).ap()
    y2eh = y2e.tensor

    with tile.TileContext(nc) as tc:
        with (
            tc.tile_pool(name="bands", bufs=1) as bpool,
            tc.tile_pool(name="warm", bufs=1) as wpool,
            tc.tile_pool(name="xin", bufs=5) as xpool,
            tc.tile_pool(name="edge", bufs=3) as epool,
            tc.tile_pool(name="out", bufs=3) as opool,
            tc.tile_pool(name="oeg", bufs=2) as oegpool,
            tc.tile_pool(name="psum", bufs=6, space="PSUM") as ppool,
            tc.tile_pool(name="psumE", bufs=2, space="PSUM") as pegpool,
        ):
            # PE p-state warmup on a zeroed scratch tile.
            wscr = wpool.tile([128, 260], F16, tag="wsrc")
            nc.vector.memset(wscr[:], 0.0)
            WPm = pegpool.tile([GMAX * ME, W], F32, tag="PE")
            for _ in range(N_WARM):
                nc.tensor.matmul(
                    WPm[:GMAX * ME, :256],
                    wscr[:128, :GMAX * ME],
                    wscr[:128, 2:258],
                    start=True,
                    stop=True,
                )

            # Two HWDGE rings (SP + ACT) carry loads only; alternate the big
            # per-image loads across rings by image parity.
            dma_engines = [nc.sync, nc.scalar]

            # img0 block 0 and the band go first, on opposite rings, so the
            # first matmul's operands land as early as possible.
            xt0 = xpool.tile([128, NB, WP], F16, name="xt0")
            nc.sync.dma_start(out=xt0[:, 0, :], in_=x2[0, :, 0, :])
            bt = bpool.tile([MB + 4, KS, MB], F16, tag="band")
            nc.scalar.dma_start(out=bt[:], in_=bm[:])
            nc.sync.dma_start(out=xt0[:, 1:, :], in_=x2[0, :, 1:, :])
            bet = bpool.tile([GMAX * KE, KS, GMAX * ME], F16, tag="bandE")
            nc.scalar.dma_start(out=bet[:], in_=bme[:])

            # Per-group edge-input tiles, partition = (img-in-group, k).
            xe_tiles = [
                epool.tile([GMAX * KE, WP], F16, tag="xe", name=f"xe{g}")
                for g in range(3)
            ]

            def edge_group(base, G):
                KEg, MEg = G * KE, G * ME
                P = pegpool.tile([GMAX * ME, W], F32, tag="PE")
                for j in range(KS):
                    nc.tensor.matmul(
                        P[:MEg, :],
                        bet[:KEg, j, :MEg],
                        xe_tiles[base // GMAX][:KEg, j:j + W],
                        start=(j == 0),
                        stop=(j == KS - 1),
                    )
                oeg = oegpool.tile([GMAX * ME, W], F16, tag="oe")
                nc.vector.tensor_copy(oeg[:MEg, :], P[:MEg, :])
                nc.gpsimd.dma_start(
                    out=bass.AP(
                        y2eh,
                        base * ME * W,
                        [[ME * W, G], [W, ME], [1, W]],
                    ),
                    in_=oeg[:MEg, :],
                )

            def xe_load(i, eng):
                gi, slot = divmod(i, GMAX)
                eng.dma_start(
                    out=xe_tiles[gi][slot * KE:(slot + 1) * KE, :],
                    in_=xe[i],
                )

            ot = None
            for img in range(IMGS_PER_CORE):
                eng_a = dma_engines[img % 2]
                eng_b = dma_engines[(img + 1) % 2]
                if img == 0:
                    xt = xt0
                else:
                    xt = xpool.tile([128, NB, WP], F16)
                    eng_a.dma_start(out=xt[:], in_=x2[img])
                # Edge rows for images 14/15 prefetch two iterations early
                # so the tail edge group can run at img13 and its store
                # drains during the last two images' compute.
                if img <= 11:
                    xe_load(img, eng_b)
                elif img == 12:
                    xe_load(12, eng_b)
                    xe_load(14, eng_b)
                elif img == 13:
                    xe_load(13, eng_b)
                    xe_load(15, eng_b)

                pair, half = divmod(img, 2)
                if half == 0:
                    ot = opool.tile([MB, 2, NB, W], F16, tag="o")

                for q in range(NB):
                    P = ppool.tile([MB, W], F32, tag="P")
                    for j in range(KS):
                        nc.tensor.matmul(
                            P[:MB, :],
                            bt[:128, j, :MB],
                            xt[:128, q, j:j + W],
                            start=(j == 0),
                            stop=(j == KS - 1),
                        )
                    nc.vector.tensor_copy(ot[:MB, half, q, :], P[:MB, :])

                # Stores, all on the SWDGE queue. Pairs (8 KB descriptors)
                # until img11; imgs 12/13 store alone at their own ends so
                # the dispatcher isn't end-loaded; the (14,15) pair is the
                # single post-compute store.
                if img in (12, 13):
                    nc.gpsimd.dma_start(out=y3[pair, :, half], in_=ot[:, half])
                elif half == 1:
                    nc.gpsimd.dma_start(out=y3[pair], in_=ot[:])

                if img == 5:
                    edge_group(0, 6)
                elif img == 11:
                    edge_group(6, 6)
                elif img == 13:
                    edge_group(12, 4)

    nc.compile()
    return nc


def kernel(X, kernel, stride, padding):
    assert int(stride) == 1 and int(padding) == 2
    X = np.asarray(X, dtype=np.float32)
    B, C, HH, WW = X.shape
    assert (B * C, HH, WW) == (N_CORES * IMGS_PER_CORE, H, W)

    if "nc" not in _CACHE:
        _CACHE["nc"] = build_nc()
    nc = _CACHE["nc"]

    band, bande = build_bands(kernel)
    Xp = np.zeros((N_CORES, IMGS_PER_CORE, HP, WP), dtype=np.float16)
    Xp[:, :, 2:2 + H, 2:2 + W] = X.reshape(N_CORES, IMGS_PER_CORE, H, W)
    rows = np.arange(128)[:, None] + (np.arange(NB) * MB)[None, :]  # [128, 4]
    x2 = Xp[:, :, rows, :]                     # [cores, imgs, 128, 4, 516]
    xe = Xp[:, :, NB * MB:NB * MB + KE, :]     # [cores, imgs, 20, 516]
    in_maps = [
        {"x2": np.ascontiguousarray(x2[c]), "xe": np.ascontiguousarray(xe[c]),
         "bm": band, "bme": bande}
        for c in range(N_CORES)
    ]
    res = run_bass_kernel_spmd(
        nc, in_maps, core_ids=list(range(N_CORES)), **_CACHE.get("run_kwargs", {})
    )
    _CACHE["last_results"] = res
    yu = np.stack([res.results[c]["y3"] for c in range(N_CORES)], axis=0)
    ye = np.stack([res.results[c]["y2e"] for c in range(N_CORES)], axis=0)
    # y3[pair, p, i2, q, w] holds output row q*124 + p of image 2*pair + i2.
    yu = yu.transpose(0, 1, 3, 4, 2, 5).reshape(
        N_CORES, IMGS_PER_CORE, NB * MB, W
    )
    out = np.concatenate([yu, ye], axis=2)     # [cores, imgs, 512, 512]
    return out.reshape(B, C, HH, WW).astype(np.float32)


# revision 17
# speedup vs baseline: 1.0499x; 1.0499x over previous
"""Depthwise 5x5 correlation (stride 1, pad 2) over X[4, 32, 512, 512] fp32,
with a single shared [5, 5] kernel, on 8 Trainium2 NeuronCores.

Strategy (pure data parallel): the 4*32 = 128 images are split 16 per core.
The input is zero-padded host-side to [516, 516], so on device the conv
decomposes per kernel column j:
    O[h, w] = sum_j C_j[h, w],   C_j[h, w] = sum_k B_j[k, h] X'[h + k, w + j]
where B_j is a banded-Toeplitz stationary matrix (B_j[k, m] = kernel[k - m,
j]); one TensorE matmul per (row-block, j), five j's accumulating into one
PSUM bank, the W shift folded into the rhs read offset. The operand path is
fp16 end to end (PSUM accumulates fp32); fp16's 2^-11 quantization is far
inside the 2e-2 gate (fp8 was measured at 4e-2 — dead).

H tiles into 4 blocks of 124 output rows (input rows q*124 + [0, 128)). The
16-row tails of all images run as block-diagonal "edge group" matmuls
packing 6 images per stationary (K = 6*20, M = 6*16): 15 matmuls replace 80.
The last group's matmuls run before the last image's uniform blocks so the
run doesn't end on edge work.

DMA facts this kernel is shaped around (measured): a descriptor covers one
SBUF-partition run x one contiguous DRAM run; the SWDGE queue dispatches
~36 ns/descriptor over 16 engines; HWDGE queues are fine for loads but
execute STORES on a single engine at ~330 ns/descriptor (poison). So:
 - loads ride the two HWDGE rings with block-interleaved DRAM layout
   x2[img, k, q, w] -> 4 KB descriptors, one DMA per image;
 - all stores ride SWDGE. Output DRAM layout y3[pair, p, i2, q, w] packs
   TWO images per partition run -> 8 KB descriptors, one store per pair;
 - the final image instead casts in 4 partition strips and stores each
   strip separately (31 descriptors apiece) so the run's last dependency
   chain is cast(0.2us) -> push -> ~1.1us dispatch.

A PE p-state warmup (20 dummy matmuls on a zeroed scratch tile) runs during
the initial DMA fill: the PE clock needs ~6 us of continuous work to ramp
0.65 -> 2.4 GHz, and real matmuls then run at full rate from the start.
"""

import numpy as np

import concourse.bacc as bacc
import concourse.bass as bass
import concourse.mybir as mybir
import concourse.tile as tile
from concourse.bass_utils import run_bass_kernel_spmd

F32 = mybir.dt.float32
F16 = mybir.dt.float16

N_CORES = 8
IMGS_PER_CORE = 16
H = W = 512
HP = H + 4
WP = W + 4
KS = 5

NB = 4           # uniform row blocks per image
MB = 124         # output rows per uniform block
ME = 16          # output rows in the edge block (rows 496..512)
KE = ME + KS - 1  # padded input rows the edge block reads

GMAX = 6         # edge-group packing factor
N_WARM = 20
NPAIR = IMGS_PER_CORE // 2

_CACHE = {}


def build_bands(kern):
    """kern: [5, 5] -> banded-Toeplitz stationaries, fp16.

    Returns (B, BE): B[128, 5, 124] uniform-block bands with
    B[k, j, m] = kern[k - m, j] for k - m in [0, 5); BE[120, 5, 96]
    block-diagonal edge bands packing GMAX images."""
    kern = np.asarray(kern, dtype=np.float32)
    B = np.zeros((MB + 4, KS, MB), dtype=np.float32)
    k_idx = np.arange(MB + 4)[:, None]
    m_idx = np.arange(MB)[None, :]
    tap = k_idx - m_idx
    valid = (tap >= 0) & (tap < KS)
    kk, mm = np.nonzero(valid)
    for j in range(KS):
        B[kk, j, mm] = kern[tap[kk, mm], j]

    BE = np.zeros((GMAX * KE, KS, GMAX * ME), dtype=np.float32)
    for g in range(GMAX):
        BE[g * KE:(g + 1) * KE, :, g * ME:(g + 1) * ME] = B[:KE, :, :ME]
    return B.astype(np.float16), BE.astype(np.float16)


def build_nc():
    nc = bacc.Bacc("TRN2", target_bir_lowering=False, debug=False)

    x2 = nc.dram_tensor(
        "x2", [IMGS_PER_CORE, 128, NB, WP], F16, kind="ExternalInput"
    ).ap()
    xe = nc.dram_tensor(
        "xe", [IMGS_PER_CORE, KE, WP], F16, kind="ExternalInput"
    ).ap()
    bm = nc.dram_tensor("bm", [MB + 4, KS, MB], F16, kind="ExternalInput").ap()
    bme = nc.dram_tensor(
        "bme", [GMAX * KE, KS, GMAX * ME], F16, kind="ExternalInput"
    ).ap()
    y3 = nc.dram_tensor(
        "y3", [NPAIR, MB, 2, NB, W], F16, kind="ExternalOutput"
    ).ap()
    y2e = nc.dram_tensor(
        "y2e", [IMGS_PER_CORE, ME, W], F16, kind="ExternalOutput"
    ).ap()
    y2eh = y2e.tensor

    with tile.TileContext(nc) as tc:
        with (
            tc.tile_pool(name="bands", bufs=1) as bpool,
            tc.tile_pool(name="warm", bufs=1) as wpool,
            tc.tile_pool(name="xin", bufs=5) as xpool,
            tc.tile_pool(name="edge", bufs=3) as epool,
            tc.tile_pool(name="out", bufs=3) as opool,
            tc.tile_pool(name="oeg", bufs=2) as oegpool,
            tc.tile_pool(name="psum", bufs=6, space="PSUM") as ppool,
            tc.tile_pool(name="psumE", bufs=2, space="PSUM") as pegpool,
        ):
            # PE p-state warmup on a zeroed scratch tile.
            wscr = wpool.tile([128, 260], F16, tag="wsrc")
            nc.vector.memset(wscr[:], 0.0)
            WPm = pegpool.tile([GMAX * ME, W], F32, tag="PE")
            for _ in range(N_WARM):
                nc.tensor.matmul(
                    WPm[:GMAX * ME, :256],
                    wscr[:128, :GMAX * ME],
                    wscr[:128, 2:258],
                    start=True,
                    stop=True,
                )

            # Two HWDGE rings (SP + ACT) carry loads only; alternate the big
            # per-image loads across rings by image parity.
            dma_engines = [nc.sync, nc.scalar]

            # img0 block 0 and the band go first, on opposite rings, so the
            # first matmul's operands land as early as possible.
            xt0 = xpool.tile([128, NB, WP], F16, name="xt0")
            nc.sync.dma_start(out=xt0[:, 0, :], in_=x2[0, :, 0, :])
            bt = bpool.tile([MB + 4, KS, MB], F16, tag="band")
            nc.scalar.dma_start(out=bt[:], in_=bm[:])
            nc.sync.dma_start(out=xt0[:, 1:, :], in_=x2[0, :, 1:, :])
            bet = bpool.tile([GMAX * KE, KS, GMAX * ME], F16, tag="bandE")
            nc.scalar.dma_start(out=bet[:], in_=bme[:])

            # Per-group edge-input tiles, partition = (img-in-group, k).
            xe_tiles = [
                epool.tile([GMAX * KE, WP], F16, tag="xe", name=f"xe{g}")
                for g in range(3)
            ]

            def edge_group(base, G):
                KEg, MEg = G * KE, G * ME
                P = pegpool.tile([GMAX * ME, W], F32, tag="PE")
                for j in range(KS):
                    nc.tensor.matmul(
                        P[:MEg, :],
                        bet[:KEg, j, :MEg],
                        xe_tiles[base // GMAX][:KEg, j:j + W],
                        start=(j == 0),
                        stop=(j == KS - 1),
                    )
                oeg = oegpool.tile([GMAX * ME, W], F16, tag="oe")
                nc.vector.tensor_copy(oeg[:MEg, :], P[:MEg, :])
                nc.gpsimd.dma_start(
                    out=bass.AP(
                        y2eh,
                        base * ME * W,
                        [[ME * W, G], [W, ME], [1, W]],
                    ),
                    in_=oeg[:MEg, :],
                )

            ot = None
            for img in range(IMGS_PER_CORE):
                gi, slot = divmod(img, GMAX)
                last = img == IMGS_PER_CORE - 1
                eng_a = dma_engines[img % 2]
                eng_b = dma_engines[(img + 1) % 2]
                if img == 0:
                    xt = xt0
                    eng_b.dma_start(
                        out=xe_tiles[gi][slot * KE:(slot + 1) * KE, :],
                        in_=xe[img],
                    )
                elif last:
                    # The tail edge group (images 12-15) only needs this
                    # image's edge rows: land them first and run the group's
                    # matmuls before the uniform blocks.
                    xt = xpool.tile([128, NB, WP], F16)
                    eng_b.dma_start(
                        out=xe_tiles[gi][slot * KE:(slot + 1) * KE, :],
                        in_=xe[img],
                    )
                    eng_a.dma_start(out=xt[:], in_=x2[img])
                    edge_group(12, 4)
                else:
                    xt = xpool.tile([128, NB, WP], F16)
                    eng_a.dma_start(out=xt[:], in_=x2[img])
                    eng_b.dma_start(
                        out=xe_tiles[gi][slot * KE:(slot + 1) * KE, :],
                        in_=xe[img],
                    )

                pair, half = divmod(img, 2)
                if half == 0:
                    ot = opool.tile([MB, 2, NB, W], F16, tag="o")

                if last:
                    # Final image: store blocks 0-2 as soon as their casts
                    # land (their dispatch overlaps block 3's matmuls), so
                    # the run's last chain is cast -> push -> one 124-
                    # descriptor piece.
                    for q in range(NB):
                        P = ppool.tile([MB, W], F32, tag="P")
                        for j in range(KS):
                            nc.tensor.matmul(
                                P[:MB, :],
                                bt[:128, j, :MB],
                                xt[:128, q, j:j + W],
                                start=(j == 0),
                                stop=(j == KS - 1),
                            )
                        nc.vector.tensor_copy(ot[:MB, half, q, :], P[:MB, :])
                        if q == NB - 2:
                            nc.gpsimd.dma_start(
                                out=y3[pair, :, half, :NB - 1],
                                in_=ot[:, half, :NB - 1],
                            )
                    nc.gpsimd.dma_start(
                        out=y3[pair, :, half, NB - 1:],
                        in_=ot[:, half, NB - 1:],
                    )
                else:
                    for q in range(NB):
                        P = ppool.tile([MB, W], F32, tag="P")
                        for j in range(KS):
                            nc.tensor.matmul(
                                P[:MB, :],
                                bt[:128, j, :MB],
                                xt[:128, q, j:j + W],
                                start=(j == 0),
                                stop=(j == KS - 1),
                            )
                        nc.vector.tensor_copy(ot[:MB, half, q, :], P[:MB, :])
                    if half == 1:
                        # One 8 KB-descriptor store per image pair.
                        nc.gpsimd.dma_start(out=y3[pair], in_=ot[:])
                    elif img == IMGS_PER_CORE - 2:
                        # img14 goes alone: its pair partner is the piece-
                        # stored final image.
                        nc.gpsimd.dma_start(out=y3[pair, :, 0], in_=ot[:, 0])

                if img == 5:
                    edge_group(0, 6)
                elif img == 11:
                    edge_group(6, 6)

    nc.compile()
    return nc


def kernel(X, kernel, stride, padding):
    assert int(stride) == 1 and int(padding) == 2
    X = np.asarray(X, dtype=np.float32)
    B, C, HH, WW = X.shape
    assert (B * C, HH, WW) == (N_CORES * IMGS_PER_CORE, H, W)

    if "nc" not in _CACHE:
        _CACHE["nc"] = build_nc()
    nc = _CACHE["nc"]

    band, bande = build_bands(kernel)
    Xp = np.zeros((N_CORES, IMGS_PER_CORE, HP, WP), dtype=np.float16)
    Xp[:, :, 2:2 + H, 2:2 + W] = X.reshape(N_CORES, IMGS_PER_CORE, H, W)
    rows = np.arange(128)[:, None] + (np.arange(NB) * MB)[None, :]  # [128, 4]
    x2 = Xp[:, :, rows, :]                     # [cores, imgs, 128, 4, 516]
    xe = Xp[:, :, NB * MB:NB * MB + KE, :]     # [cores, imgs, 20, 516]
    in_maps = [
        {"x2": np.ascontiguousarray(x2[c]), "xe": np.ascontiguousarray(xe[c]),
         "bm": band, "bme": bande}
        for c in range(N_CORES)
    ]
    res = run_bass_kernel_spmd(
        nc, in_maps, core_ids=list(range(N_CORES)), **_CACHE.get("run_kwargs", {})
    )
    _CACHE["last_results"] = res
    yu = np.stack([res.results[c]["y3"] for c in range(N_CORES)], axis=0)
    ye = np.stack([res.results[c]["y2e"] for c in range(N_CORES)], axis=0)
    # y3[pair, p, i2, q, w] holds output row q*124 + p of image 2*pair + i2.
    yu = yu.transpose(0, 1, 3, 4, 2, 5).reshape(
        N_CORES, IMGS_PER_CORE, NB * MB, W
    )
    out = np.concatenate([yu, ye], axis=2)     # [cores, imgs, 512, 512]
    return out.reshape(B, C, HH, WW).astype(np.float32)


# revision 18
# speedup vs baseline: 1.0576x; 1.0074x over previous
"""Depthwise 5x5 correlation (stride 1, pad 2) over X[4, 32, 512, 512] fp32,
with a single shared [5, 5] kernel, on 8 Trainium2 NeuronCores.

Strategy (pure data parallel): the 4*32 = 128 images are split 16 per core.
The input is zero-padded host-side to [516, 516], so on device the conv
decomposes per kernel column j:
    O[h, w] = sum_j C_j[h, w],   C_j[h, w] = sum_k B_j[k, h] X'[h + k, w + j]
where B_j is a banded-Toeplitz stationary matrix (B_j[k, m] = kernel[k - m,
j]); one TensorE matmul per (row-block, j), five j's accumulating into one
PSUM bank, the W shift folded into the rhs read offset. The operand path is
fp16 end to end (PSUM accumulates fp32); fp16's 2^-11 quantization is far
inside the 2e-2 gate (fp8 was measured at 4e-2 — dead).

H tiles into 4 blocks of 124 output rows (input rows q*124 + [0, 128)). The
16-row tails of all images run as block-diagonal "edge group" matmuls
packing 6 images per stationary (K = 6*20, M = 6*16): 15 matmuls replace 80.
The last group's matmuls run before the last image's uniform blocks so the
run doesn't end on edge work.

DMA facts this kernel is shaped around (measured): a descriptor covers one
SBUF-partition run x one contiguous DRAM run; the SWDGE queue dispatches
~36 ns/descriptor over 16 engines; HWDGE queues are fine for loads but
execute STORES on a single engine at ~330 ns/descriptor (poison). So:
 - loads ride the two HWDGE rings with block-interleaved DRAM layout
   x2[img, k, q, w] -> 4 KB descriptors, one DMA per image;
 - all stores ride SWDGE. Output DRAM layout y3[pair, p, i2, q, w] packs
   TWO images per partition run -> 8 KB descriptors, one store per pair;
 - the final image instead casts in 4 partition strips and stores each
   strip separately (31 descriptors apiece) so the run's last dependency
   chain is cast(0.2us) -> push -> ~1.1us dispatch.

A PE p-state warmup (20 dummy matmuls on a zeroed scratch tile) runs during
the initial DMA fill: the PE clock needs ~6 us of continuous work to ramp
0.65 -> 2.4 GHz, and real matmuls then run at full rate from the start.
"""

import numpy as np

import concourse.bacc as bacc
import concourse.bass as bass
import concourse.mybir as mybir
import concourse.tile as tile
from concourse.bass_utils import run_bass_kernel_spmd

F32 = mybir.dt.float32
F16 = mybir.dt.float16

N_CORES = 8
IMGS_PER_CORE = 16
H = W = 512
HP = H + 4
WP = W + 4
KS = 5

NB = 4           # uniform row blocks per image
MB = 124         # output rows per uniform block
ME = 16          # output rows in the edge block (rows 496..512)
KE = ME + KS - 1  # padded input rows the edge block reads

GMAX = 6         # edge-group packing factor
N_WARM = 20
NPAIR = IMGS_PER_CORE // 2

_CACHE = {}


def build_bands(kern):
    """kern: [5, 5] -> banded-Toeplitz stationaries, fp16.

    Returns (B, BE): B[128, 5, 124] uniform-block bands with
    B[k, j, m] = kern[k - m, j] for k - m in [0, 5); BE[120, 5, 96]
    block-diagonal edge bands packing GMAX images."""
    kern = np.asarray(kern, dtype=np.float32)
    B = np.zeros((MB + 4, KS, MB), dtype=np.float32)
    k_idx = np.arange(MB + 4)[:, None]
    m_idx = np.arange(MB)[None, :]
    tap = k_idx - m_idx
    valid = (tap >= 0) & (tap < KS)
    kk, mm = np.nonzero(valid)
    for j in range(KS):
        B[kk, j, mm] = kern[tap[kk, mm], j]

    BE = np.zeros((GMAX * KE, KS, GMAX * ME), dtype=np.float32)
    for g in range(GMAX):
        BE[g * KE:(g + 1) * KE, :, g * ME:(g + 1) * ME] = B[:KE, :, :ME]
    return B.astype(np.float16), BE.astype(np.float16)


def build_nc():
    nc = bacc.Bacc("TRN2", target_bir_lowering=False, debug=False)

    x2 = nc.dram_tensor(
        "x2", [IMGS_PER_CORE, 128, NB, WP], F16, kind="ExternalInput"
    ).ap()
    xe = nc.dram_tensor(
        "xe", [IMGS_PER_CORE, KE, WP], F16, kind="ExternalInput"
    ).ap()
    bm = nc.dram_tensor("bm", [MB + 4, KS, MB], F16, kind="ExternalInput").ap()
    bme = nc.dram_tensor(
        "bme", [GMAX * KE, KS, GMAX * ME], F16, kind="ExternalInput"
    ).ap()
    y3 = nc.dram_tensor(
        "y3", [NPAIR, MB, 2, NB, W], F16, kind="ExternalOutput"
    ).ap()
    y2e = nc.dram_tensor(
        "y2e", [IMGS_PER_CORE, ME, W], F16, kind="ExternalOutput"
    ).ap()
    y2eh = y2e.tensor

    with tile.TileContext(nc) as tc:
        with (
            tc.tile_pool(name="bands", bufs=1) as bpool,
            tc.tile_pool(name="warm", bufs=1) as wpool,
            tc.tile_pool(name="xin", bufs=5) as xpool,
            tc.tile_pool(name="edge", bufs=3) as epool,
            tc.tile_pool(name="out", bufs=3) as opool,
            tc.tile_pool(name="oeg", bufs=2) as oegpool,
            tc.tile_pool(name="psum", bufs=6, space="PSUM") as ppool,
            tc.tile_pool(name="psumE", bufs=2, space="PSUM") as pegpool,
        ):
            # PE p-state warmup on a zeroed scratch tile. The memset runs on
            # gpsimd, whose preamble retires earliest, so the PE clock ramp
            # starts as soon as possible.
            wscr = wpool.tile([128, 260], F16, tag="wsrc")
            nc.gpsimd.memset(wscr[:], 0.0)
            WPm = pegpool.tile([GMAX * ME, W], F32, tag="PE")
            for _ in range(N_WARM):
                nc.tensor.matmul(
                    WPm[:GMAX * ME, :256],
                    wscr[:128, :GMAX * ME],
                    wscr[:128, 2:258],
                    start=True,
                    stop=True,
                )

            # Two HWDGE rings (SP + ACT) carry loads only; alternate the big
            # per-image loads across rings by image parity.
            dma_engines = [nc.sync, nc.scalar]

            # img0 block 0 and the band go first, on opposite rings, so the
            # first matmul's operands land as early as possible.
            xt0 = xpool.tile([128, NB, WP], F16, name="xt0")
            nc.sync.dma_start(out=xt0[:, 0, :], in_=x2[0, :, 0, :])
            bt = bpool.tile([MB + 4, KS, MB], F16, tag="band")
            nc.scalar.dma_start(out=bt[:], in_=bm[:])
            nc.sync.dma_start(out=xt0[:, 1:, :], in_=x2[0, :, 1:, :])
            bet = bpool.tile([GMAX * KE, KS, GMAX * ME], F16, tag="bandE")
            nc.scalar.dma_start(out=bet[:], in_=bme[:])

            # Per-group edge-input tiles, partition = (img-in-group, k).
            xe_tiles = [
                epool.tile([GMAX * KE, WP], F16, tag="xe", name=f"xe{g}")
                for g in range(3)
            ]

            def edge_group(base, G):
                KEg, MEg = G * KE, G * ME
                P = pegpool.tile([GMAX * ME, W], F32, tag="PE")
                for j in range(KS):
                    nc.tensor.matmul(
                        P[:MEg, :],
                        bet[:KEg, j, :MEg],
                        xe_tiles[base // GMAX][:KEg, j:j + W],
                        start=(j == 0),
                        stop=(j == KS - 1),
                    )
                oeg = oegpool.tile([GMAX * ME, W], F16, tag="oe")
                nc.vector.tensor_copy(oeg[:MEg, :], P[:MEg, :])
                nc.gpsimd.dma_start(
                    out=bass.AP(
                        y2eh,
                        base * ME * W,
                        [[ME * W, G], [W, ME], [1, W]],
                    ),
                    in_=oeg[:MEg, :],
                )

            ot = None
            for img in range(IMGS_PER_CORE):
                gi, slot = divmod(img, GMAX)
                last = img == IMGS_PER_CORE - 1
                eng_a = dma_engines[img % 2]
                eng_b = dma_engines[(img + 1) % 2]
                if img == 0:
                    xt = xt0
                    eng_b.dma_start(
                        out=xe_tiles[gi][slot * KE:(slot + 1) * KE, :],
                        in_=xe[img],
                    )
                elif last:
                    # The tail edge group (images 12-15) only needs this
                    # image's edge rows: land them first and run the group's
                    # matmuls before the uniform blocks.
                    xt = xpool.tile([128, NB, WP], F16)
                    eng_b.dma_start(
                        out=xe_tiles[gi][slot * KE:(slot + 1) * KE, :],
                        in_=xe[img],
                    )
                    eng_a.dma_start(out=xt[:], in_=x2[img])
                    edge_group(12, 4)
                else:
                    xt = xpool.tile([128, NB, WP], F16)
                    eng_a.dma_start(out=xt[:], in_=x2[img])
                    eng_b.dma_start(
                        out=xe_tiles[gi][slot * KE:(slot + 1) * KE, :],
                        in_=xe[img],
                    )

                pair, half = divmod(img, 2)
                if half == 0:
                    ot = opool.tile([MB, 2, NB, W], F16, tag="o")

                if last:
                    # Final image: store blocks 0-2 as soon as their casts
                    # land (their dispatch overlaps block 3's matmuls), so
                    # the run's last chain is cast -> push -> one 124-
                    # descriptor piece.
                    for q in range(NB):
                        P = ppool.tile([MB, W], F32, tag="P")
                        for j in range(KS):
                            nc.tensor.matmul(
                                P[:MB, :],
                                bt[:128, j, :MB],
                                xt[:128, q, j:j + W],
                                start=(j == 0),
                                stop=(j == KS - 1),
                            )
                        nc.vector.tensor_copy(ot[:MB, half, q, :], P[:MB, :])
                        if q == NB - 2:
                            nc.gpsimd.dma_start(
                                out=y3[pair, :, half, :NB - 1],
                                in_=ot[:, half, :NB - 1],
                            )
                    nc.gpsimd.dma_start(
                        out=y3[pair, :, half, NB - 1:],
                        in_=ot[:, half, NB - 1:],
                    )
                else:
                    for q in range(NB):
                        P = ppool.tile([MB, W], F32, tag="P")
                        for j in range(KS):
                            nc.tensor.matmul(
                                P[:MB, :],
                                bt[:128, j, :MB],
                                xt[:128, q, j:j + W],
                                start=(j == 0),
                                stop=(j == KS - 1),
                            )
                        nc.vector.tensor_copy(ot[:MB, half, q, :], P[:MB, :])
                    if half == 1:
                        # One 8 KB-descriptor store per image pair.
                        nc.gpsimd.dma_start(out=y3[pair], in_=ot[:])
                    elif img == IMGS_PER_CORE - 2:
                        # img14 goes alone: its pair partner is the piece-
                        # stored final image.
                        nc.gpsimd.dma_start(out=y3[pair, :, 0], in_=ot[:, 0])

                if img == 5:
                    edge_group(0, 6)
                elif img == 11:
                    edge_group(6, 6)

    nc.compile()
    return nc


def kernel(X, kernel, stride, padding):
    assert int(stride) == 1 and int(padding) == 2
    X = np.asarray(X, dtype=np.float32)
    B, C, HH, WW = X.shape
    assert (B * C, HH, WW) == (N_CORES * IMGS_PER_CORE, H, W)

    if "nc" not in _CACHE:
        _CACHE["nc"] = build_nc()
    nc = _CACHE["nc"]

    band, bande = build_bands(kernel)
    Xp = np.zeros((N_CORES, IMGS_PER_CORE, HP, WP), dtype=np.float16)
    Xp[:, :, 2:2 + H, 2:2 + W] = X.reshape(N_CORES, IMGS_PER_CORE, H, W)
    rows = np.arange(128)[:, None] + (np.arange(NB) * MB)[None, :]  # [128, 4]
    x2 = Xp[:, :, rows, :]                     # [cores, imgs, 128, 4, 516]
    xe = Xp[:, :, NB * MB:NB * MB + KE, :]     # [cores, imgs, 20, 516]
    in_maps = [
        {"x2": np.ascontiguousarray(x2[c]), "xe": np.ascontiguousarray(xe[c]),
         "bm": band, "bme": bande}
        for c in range(N_CORES)
    ]
    res = run_bass_kernel_spmd(
        nc, in_maps, core_ids=list(range(N_CORES)), **_CACHE.get("run_kwargs", {})
    )
    _CACHE["last_results"] = res
    yu = np.stack([res.results[c]["y3"] for c in range(N_CORES)], axis=0)
    ye = np.stack([res.results[c]["y2e"] for c in range(N_CORES)], axis=0)
    # y3[pair, p, i2, q, w] holds output row q*124 + p of image 2*pair + i2.
    yu = yu.transpose(0, 1, 3, 4, 2, 5).reshape(
        N_CORES, IMGS_PER_CORE, NB * MB, W
    )
    out = np.concatenate([yu, ye], axis=2)     # [cores, imgs, 512, 512]
    return out.reshape(B, C, HH, WW).astype(np.float32)
